# revision 3
# baseline (speedup 1.0000x reference)
"""Trainium2 Bass kernel for nn_ConvThreshold: 5x5 scale-adaptive Gaussian
blur (per-pixel bandwidth) + ReLU front + threshold mask.

conv[p] = sum_{dy,dx in [-2,2]} relu(x)[p+(dy,dx)] * t[p]^(dy^2+dx^2)
with t[p] = exp(-1/(2*scale[p]^2 + eps)); mask = conv >= 0.5.

Decomposition: group the 25 taps into 6 rings by r2 = dy^2+dx^2 in
{0,1,2,4,5,8}; conv = R0 + sum_k t^k * Rk. Ring sums are built on the
TensorEngine (fp16 1cyc/col matmuls, fp32 PSUM accumulate): vertical taps
via banded lhsT (+K=4 halo matmuls), horizontal taps via free-dim-shifted
identity matmuls. Diagonal rings R2/R5/R8 come from free-axis shifts of the
vertical sums V1/V2 on DVE/GPSIMD. Per-pixel weights t^k on ScalarE (exp),
products on DVE, final 6-term sum back on TensorE into PSUM.

Sharding: 8 cores = 4 images x 2 vertical halves (384 rows each, +-2 row
halo). Full inputs in, full outputs out.
"""

import sys

sys.path.insert(0, "/opt/trn_rl_repo")

from contextlib import ExitStack

import numpy as np

import concourse.bass as bass
import concourse.tile as tile
from concourse import bacc, mybir
from concourse.bass_utils import run_bass_kernel_spmd

F32 = mybir.dt.float32
F16 = mybir.dt.float16

B, H, W = 4, 768, 768
NCORES = 8
SLAB = H // 2          # rows per core
NSTRIPES = SLAB // 128  # 3
PAD = 2
WP = W + 2 * PAD       # 772
COLH = [(0, 512), (512, 256)]  # PSUM col-halves
SQRT2 = float(np.sqrt(2.0))
RINGS = [1, 2, 4, 5, 8]

_CACHE = {}


def _consts():
    ident = np.eye(128, dtype=np.float16)
    b1 = np.zeros((128, 128), dtype=np.float16)
    b2 = np.zeros((128, 128), dtype=np.float16)
    for m in range(128):
        for d in (-1, 1):
            if 0 <= m + d < 128:
                b1[m + d, m] = 1.0
        for d in (-2, 2):
            if 0 <= m + d < 128:
                b2[m + d, m] = 1.0
    hv1 = np.zeros((4, 128), dtype=np.float16)
    hv1[1, 0] = 1.0
    hv1[2, 127] = 1.0
    hv2 = np.zeros((4, 128), dtype=np.float16)
    hv2[0, 0] = 1.0
    hv2[1, 1] = 1.0
    hv2[2, 126] = 1.0
    hv2[3, 127] = 1.0
    return {"w_id": ident, "w_b1": b1, "w_b2": b2, "w_hv1": hv1, "w_hv2": hv2}


def _build(repeat: int = 1):
    nc = bacc.Bacc(
        "TRN2",
        target_bir_lowering=False,
        debug=False,
        enable_asserts=True,
        num_devices=NCORES,
    )
    xin = nc.dram_tensor("xin", [SLAB + 4, W], F32, kind="ExternalInput").ap()
    sin = nc.dram_tensor("sin", [SLAB, W], F32, kind="ExternalInput").ap()
    cd = {
        name: nc.dram_tensor(name, list(arr.shape), F16, kind="ExternalInput").ap()
        for name, arr in _consts().items()
    }
    conv_d = nc.dram_tensor("conv", [SLAB, W], F32, kind="ExternalOutput").ap()
    mask_d = nc.dram_tensor("mask", [SLAB, W], F32, kind="ExternalOutput").ap()

    with tile.TileContext(nc, trace_sim=False) as tc, ExitStack() as ctx:
        sb = ctx.enter_context(tc.tile_pool(name="sb", bufs=2))
        cb = ctx.enter_context(tc.tile_pool(name="cb", bufs=1))
        ps = ctx.enter_context(tc.tile_pool(name="ps", bufs=1, space="PSUM"))

        # stationary matrices, loaded once
        wt = {}
        for name, arr in _consts().items():
            t = cb.tile(list(arr.shape), F16, tag=name)
            nc.sync.dma_start(t[:], cd[name][:])
            wt[name] = t

        for st in range(NSTRIPES * repeat):
            st = st % NSTRIPES
            r0 = 128 * st  # slab row of stripe start

            # ---- input DMAs (f32) ----
            xf = sb.tile([128, W], F32, tag="xf")
            nc.sync.dma_start(xf[:], xin[r0 + 2 : r0 + 130, :])
            hf = sb.tile([4, W], F32, tag="hf")
            nc.sync.dma_start(hf[0:2, :], xin[r0 : r0 + 2, :])
            nc.sync.dma_start(hf[2:4, :], xin[r0 + 130 : r0 + 132, :])
            sf = sb.tile([128, W], F32, tag="sf")
            nc.sync.dma_start(sf[:], sin[r0 : r0 + 128, :])

            # ---- relu + cast to fp16, into column-padded tiles ----
            x16 = sb.tile([128, WP], F16, tag="x16")
            nc.gpsimd.memset(x16[:, 0:2], 0.0)
            nc.gpsimd.memset(x16[:, WP - 2 : WP], 0.0)
            nc.scalar.activation(
                x16[:, 2 : 2 + W], xf[:], mybir.ActivationFunctionType.Relu
            )
            h16 = sb.tile([4, WP], F16, tag="h16")
            nc.gpsimd.memset(h16[:, 0:2], 0.0)
            nc.gpsimd.memset(h16[:, WP - 2 : WP], 0.0)
            nc.scalar.activation(
                h16[:, 2 : 2 + W], hf[:], mybir.ActivationFunctionType.Relu
            )

            # ---- weights t^k = exp(-k/(2 s^2)) ----
            u = sb.tile([128, W], F32, tag="u")
            nc.scalar.activation(
                u[:], sf[:], mybir.ActivationFunctionType.Square, scale=SQRT2
            )
            v = sb.tile([128, W], F32, tag="v")
            nc.vector.reciprocal_approx_fast(v[:], u[:])
            g = {}
            for k in RINGS:
                gk = sb.tile([128, W], F16, tag=f"g{k}")
                nc.scalar.activation(
                    gk[:], v[:], mybir.ActivationFunctionType.Exp, scale=-float(k)
                )
                g[k] = gk

            # ---- vertical ring sums V1/V2 on PE; extend to R1/R4 ----
            v1s = sb.tile([128, WP], F16, tag="v1s")
            v2s = sb.tile([128, WP], F16, tag="v2s")
            for t_, nm in ((v1s, "v1s"), (v2s, "v2s")):
                nc.gpsimd.memset(t_[:, 0:2], 0.0)
                nc.gpsimd.memset(t_[:, WP - 2 : WP], 0.0)

            w1 = sb.tile([128, W], F16, tag="w1")
            w4 = sb.tile([128, W], F16, tag="w4")
            v1p_h = []
            v2p_h = []
            for ci, (c0, n) in enumerate(COLH):
                xc = lambda dx, c0=c0, n=n: x16[:, 2 + c0 + dx : 2 + c0 + dx + n]
                hc = lambda dx, c0=c0, n=n: h16[:, 2 + c0 + dx : 2 + c0 + dx + n]

                v1p = ps.tile([128, n], F32, tag=f"v1p{ci}")
                nc.tensor.matmul(v1p[:], wt["w_b1"][:], xc(0), start=True, stop=False)
                nc.tensor.matmul(v1p[:], wt["w_hv1"][:], hc(0), start=False, stop=True)
                v2p = ps.tile([128, n], F32, tag=f"v2p{ci}")
                nc.tensor.matmul(v2p[:], wt["w_b2"][:], xc(0), start=True, stop=False)
                nc.tensor.matmul(v2p[:], wt["w_hv2"][:], hc(0), start=False, stop=True)

                # evacuate V1/V2 (fp16) for the diagonal rings
                nc.scalar.copy(v1s[:, 2 + c0 : 2 + c0 + n], v1p[:])
                nc.scalar.copy(v2s[:, 2 + c0 : 2 + c0 + n], v2p[:])

                # R1 = V1 + x(+-1);  R4 = V2 + x(+-2)   (accumulate in place)
                nc.tensor.matmul(v1p[:], wt["w_id"][:], xc(-1), start=False, stop=False)
                nc.tensor.matmul(v1p[:], wt["w_id"][:], xc(+1), start=False, stop=True)
                nc.tensor.matmul(v2p[:], wt["w_id"][:], xc(-2), start=False, stop=False)
                nc.tensor.matmul(v2p[:], wt["w_id"][:], xc(+2), start=False, stop=True)

                # products for rings 1 and 4 straight from PSUM
                nc.vector.tensor_mul(w1[:, c0 : c0 + n], v1p[:], g[1][:, c0 : c0 + n])
                nc.vector.tensor_mul(w4[:, c0 : c0 + n], v2p[:], g[4][:, c0 : c0 + n])
                v1p_h.append(v1p)
                v2p_h.append(v2p)

            # ---- diagonal rings from V1s/V2s (free-axis shifts) ----
            r2s = sb.tile([128, W], F16, tag="r2s")
            nc.vector.tensor_add(r2s[:], v1s[:, 1 : 1 + W], v1s[:, 3 : 3 + W])
            r8s = sb.tile([128, W], F16, tag="r8s")
            nc.gpsimd.tensor_add(r8s[:], v2s[:, 0:W], v2s[:, 4 : 4 + W])
            r5a = sb.tile([128, W], F16, tag="r5a")
            nc.vector.tensor_add(r5a[:], v1s[:, 0:W], v1s[:, 4 : 4 + W])
            r5b = sb.tile([128, W], F16, tag="r5b")
            nc.vector.tensor_add(r5b[:], v2s[:, 1 : 1 + W], v2s[:, 3 : 3 + W])
            r5s = sb.tile([128, W], F16, tag="r5s")
            nc.vector.tensor_add(r5s[:], r5a[:], r5b[:])

            w2 = sb.tile([128, W], F16, tag="w2")
            nc.vector.tensor_mul(w2[:], r2s[:], g[2][:])
            w5 = sb.tile([128, W], F16, tag="w5")
            nc.vector.tensor_mul(w5[:], r5s[:], g[5][:])
            w8 = sb.tile([128, W], F16, tag="w8")
            nc.gpsimd.tensor_mul(w8[:], r8s[:], g[8][:])

            # ---- conv = x + W1 + W2 + W4 + W5 + W8 (PE accumulate) ----
            conv_sb = sb.tile([128, W], F32, tag="conv_sb")
            mask_sb = sb.tile([128, W], F32, tag="mask_sb")
            for ci, (c0, n) in enumerate(COLH):
                cp = ps.tile([128, n], F32, tag=f"cp{ci}")
                nc.tensor.matmul(
                    cp[:], wt["w_id"][:], x16[:, 2 + c0 : 2 + c0 + n],
                    start=True, stop=False,
                )
                for wk in (w1, w2, w4, w5):
                    nc.tensor.matmul(
                        cp[:], wt["w_id"][:], wk[:, c0 : c0 + n],
                        start=False, stop=False,
                    )
                nc.tensor.matmul(
                    cp[:], wt["w_id"][:], w8[:, c0 : c0 + n], start=False, stop=True
                )
                nc.scalar.copy(conv_sb[:, c0 : c0 + n], cp[:])
                nc.vector.tensor_scalar(
                    mask_sb[:, c0 : c0 + n], cp[:], 0.5, None, mybir.AluOpType.is_ge
                )

            nc.sync.dma_start(conv_d[r0 : r0 + 128, :], conv_sb[:])
            nc.sync.dma_start(mask_d[r0 : r0 + 128, :], mask_sb[:])

    nc.compile()
    return nc


def kernel(bev_map: np.ndarray, bev_scale: np.ndarray):
    assert bev_map.shape == (B, 1, H, W) and bev_scale.shape == (B, 1, H, W)
    if "nc" not in _CACHE:
        _CACHE["nc"] = _build()
    nc = _CACHE["nc"]

    consts = _consts()
    in_maps = []
    for c in range(NCORES):
        b, hh = c // 2, c % 2
        padded = np.pad(bev_map[b, 0], ((2, 2), (0, 0)))  # [772, W]
        m = {
            "xin": np.ascontiguousarray(padded[hh * SLAB : hh * SLAB + SLAB + 4]),
            "sin": np.ascontiguousarray(bev_scale[b, 0, hh * SLAB : (hh + 1) * SLAB]),
        }
        m.update(consts)
        in_maps.append(m)

    res = run_bass_kernel_spmd(nc, in_maps, list(range(NCORES))).results

    conv = np.empty((B, 1, H, W), dtype=np.float32)
    mask = np.empty((B, 1, H, W), dtype=np.float32)
    for c in range(NCORES):
        b, hh = c // 2, c % 2
        conv[b, 0, hh * SLAB : (hh + 1) * SLAB] = res[c]["conv"]
        mask[b, 0, hh * SLAB : (hh + 1) * SLAB] = res[c]["mask"]
    return conv, mask


# revision 5
# speedup vs baseline: 206.3114x; 206.3114x over previous
"""Trainium2 Bass kernel for nn_ConvThreshold: 5x5 scale-adaptive Gaussian
blur (per-pixel bandwidth) + ReLU front + threshold mask.

conv[p] = sum_{dy,dx in [-2,2]} relu(x)[p+(dy,dx)] * t[p]^(dy^2+dx^2)
with t[p] = exp(-1/(2*scale[p]^2 + eps)); mask = conv >= 0.5.

Decomposition: group the 25 taps into 6 rings by r2 = dy^2+dx^2 in
{0,1,2,4,5,8}; conv = R0 + sum_k t^k * Rk. Ring sums are built on the
TensorEngine (fp16 1cyc/col matmuls, fp32 PSUM accumulate): vertical taps
via banded lhsT (+K=4 halo matmuls), horizontal taps via free-dim-shifted
identity matmuls. Diagonal rings R2/R5/R8 come from free-axis shifts of the
vertical sums V1/V2 on DVE/GPSIMD. Per-pixel weights t^k on ScalarE (exp),
products on DVE, final 6-term sum back on TensorE into PSUM.

Sharding: 8 cores = 4 images x 2 vertical halves (384 rows each, +-2 row
halo). Full inputs in, full outputs out.
"""

import sys

sys.path.insert(0, "/opt/trn_rl_repo")

from contextlib import ExitStack

import numpy as np

import concourse.bass as bass
import concourse.tile as tile
from concourse import bacc, mybir
from concourse.bass_utils import run_bass_kernel_spmd

F32 = mybir.dt.float32
F16 = mybir.dt.float16

B, H, W = 4, 768, 768
NCORES = 8
SLAB = H // 2          # rows per core
NSTRIPES = SLAB // 128  # 3
PAD = 2
WP = W + 2 * PAD       # 772
COLH = [(0, 512), (512, 256)]  # PSUM col-halves
SQRT2 = float(np.sqrt(2.0))
RINGS = [1, 2, 4, 5, 8]

_CACHE = {}


def _consts():
    ident = np.eye(128, dtype=np.float16)
    b1 = np.zeros((128, 128), dtype=np.float16)
    b2 = np.zeros((128, 128), dtype=np.float16)
    for m in range(128):
        for d in (-1, 1):
            if 0 <= m + d < 128:
                b1[m + d, m] = 1.0
        for d in (-2, 2):
            if 0 <= m + d < 128:
                b2[m + d, m] = 1.0
    hv1 = np.zeros((4, 128), dtype=np.float16)
    hv1[1, 0] = 1.0
    hv1[2, 127] = 1.0
    hv2 = np.zeros((4, 128), dtype=np.float16)
    hv2[0, 0] = 1.0
    hv2[1, 1] = 1.0
    hv2[2, 126] = 1.0
    hv2[3, 127] = 1.0
    return {"w_id": ident, "w_b1": b1, "w_b2": b2, "w_hv1": hv1, "w_hv2": hv2}


def _build(repeat: int = 1):
    nc = bacc.Bacc(
        "TRN2",
        target_bir_lowering=False,
        debug=False,
        enable_asserts=True,
        num_devices=NCORES,
    )
    xin = nc.dram_tensor("xin", [SLAB + 4, W], F32, kind="ExternalInput").ap()
    sin = nc.dram_tensor("sin", [SLAB, W], F32, kind="ExternalInput").ap()
    cd = {
        name: nc.dram_tensor(name, list(arr.shape), F16, kind="ExternalInput").ap()
        for name, arr in _consts().items()
    }
    conv_d = nc.dram_tensor("conv", [SLAB, W], F32, kind="ExternalOutput").ap()
    mask_d = nc.dram_tensor("mask", [SLAB, W], F32, kind="ExternalOutput").ap()

    with tile.TileContext(nc, trace_sim=False) as tc, ExitStack() as ctx:
        sb = ctx.enter_context(tc.tile_pool(name="sb", bufs=2))
        cb = ctx.enter_context(tc.tile_pool(name="cb", bufs=1))
        ps = ctx.enter_context(tc.tile_pool(name="ps", bufs=1, space="PSUM"))

        # stationary matrices, loaded once
        wt = {}
        for name, arr in _consts().items():
            t = cb.tile(list(arr.shape), F16, tag=name)
            nc.sync.dma_start(t[:], cd[name][:])
            wt[name] = t

        def _body():
          for st in range(NSTRIPES):
            r0 = 128 * st  # slab row of stripe start

            # ---- input DMAs (f32) ----
            xf = sb.tile([128, W], F32, tag="xf")
            nc.sync.dma_start(xf[:], xin[r0 + 2 : r0 + 130, :])
            hf = sb.tile([4, W], F32, tag="hf")
            nc.sync.dma_start(hf[0:2, :], xin[r0 : r0 + 2, :])
            nc.sync.dma_start(hf[2:4, :], xin[r0 + 130 : r0 + 132, :])
            sf = sb.tile([128, W], F32, tag="sf")
            nc.sync.dma_start(sf[:], sin[r0 : r0 + 128, :])

            # ---- relu + cast to fp16, into column-padded tiles ----
            x16 = sb.tile([128, WP], F16, tag="x16")
            nc.gpsimd.memset(x16[:, 0:2], 0.0)
            nc.gpsimd.memset(x16[:, WP - 2 : WP], 0.0)
            nc.scalar.activation(
                x16[:, 2 : 2 + W], xf[:], mybir.ActivationFunctionType.Relu
            )
            h16 = sb.tile([4, WP], F16, tag="h16")
            nc.gpsimd.memset(h16[:, 0:2], 0.0)
            nc.gpsimd.memset(h16[:, WP - 2 : WP], 0.0)
            nc.scalar.activation(
                h16[:, 2 : 2 + W], hf[:], mybir.ActivationFunctionType.Relu
            )

            # ---- weights t^k = exp(-k/(2 s^2)) ----
            u = sb.tile([128, W], F32, tag="u")
            nc.scalar.activation(
                u[:], sf[:], mybir.ActivationFunctionType.Square, scale=SQRT2
            )
            v = sb.tile([128, W], F32, tag="v")
            nc.vector.reciprocal_approx_fast(v[:], u[:])
            g = {}
            for k in RINGS:
                gk = sb.tile([128, W], F16, tag=f"g{k}")
                nc.scalar.activation(
                    gk[:], v[:], mybir.ActivationFunctionType.Exp, scale=-float(k)
                )
                g[k] = gk

            # ---- vertical ring sums V1/V2 on PE; extend to R1/R4 ----
            v1s = sb.tile([128, WP], F16, tag="v1s")
            v2s = sb.tile([128, WP], F16, tag="v2s")
            for t_, nm in ((v1s, "v1s"), (v2s, "v2s")):
                nc.gpsimd.memset(t_[:, 0:2], 0.0)
                nc.gpsimd.memset(t_[:, WP - 2 : WP], 0.0)

            w1 = sb.tile([128, W], F16, tag="w1")
            w4 = sb.tile([128, W], F16, tag="w4")
            v1p_h = []
            v2p_h = []
            for ci, (c0, n) in enumerate(COLH):
                xc = lambda dx, c0=c0, n=n: x16[:, 2 + c0 + dx : 2 + c0 + dx + n]
                hc = lambda dx, c0=c0, n=n: h16[:, 2 + c0 + dx : 2 + c0 + dx + n]

                v1p = ps.tile([128, n], F32, tag=f"v1p{ci}")
                nc.tensor.matmul(v1p[:], wt["w_b1"][:], xc(0), start=True, stop=False)
                nc.tensor.matmul(v1p[:], wt["w_hv1"][:], hc(0), start=False, stop=True)
                v2p = ps.tile([128, n], F32, tag=f"v2p{ci}")
                nc.tensor.matmul(v2p[:], wt["w_b2"][:], xc(0), start=True, stop=False)
                nc.tensor.matmul(v2p[:], wt["w_hv2"][:], hc(0), start=False, stop=True)

                # evacuate V1/V2 (fp16) for the diagonal rings
                nc.scalar.copy(v1s[:, 2 + c0 : 2 + c0 + n], v1p[:])
                nc.scalar.copy(v2s[:, 2 + c0 : 2 + c0 + n], v2p[:])

                # R1 = V1 + x(+-1);  R4 = V2 + x(+-2)   (accumulate in place)
                nc.tensor.matmul(v1p[:], wt["w_id"][:], xc(-1), start=False, stop=False)
                nc.tensor.matmul(v1p[:], wt["w_id"][:], xc(+1), start=False, stop=True)
                nc.tensor.matmul(v2p[:], wt["w_id"][:], xc(-2), start=False, stop=False)
                nc.tensor.matmul(v2p[:], wt["w_id"][:], xc(+2), start=False, stop=True)

                # products for rings 1 and 4 straight from PSUM
                nc.vector.tensor_mul(w1[:, c0 : c0 + n], v1p[:], g[1][:, c0 : c0 + n])
                nc.vector.tensor_mul(w4[:, c0 : c0 + n], v2p[:], g[4][:, c0 : c0 + n])
                v1p_h.append(v1p)
                v2p_h.append(v2p)

            # ---- diagonal rings from V1s/V2s (free-axis shifts) ----
            r2s = sb.tile([128, W], F16, tag="r2s")
            nc.vector.tensor_add(r2s[:], v1s[:, 1 : 1 + W], v1s[:, 3 : 3 + W])
            r8s = sb.tile([128, W], F16, tag="r8s")
            nc.gpsimd.tensor_add(r8s[:], v2s[:, 0:W], v2s[:, 4 : 4 + W])
            r5a = sb.tile([128, W], F16, tag="r5a")
            nc.vector.tensor_add(r5a[:], v1s[:, 0:W], v1s[:, 4 : 4 + W])
            r5b = sb.tile([128, W], F16, tag="r5b")
            nc.vector.tensor_add(r5b[:], v2s[:, 1 : 1 + W], v2s[:, 3 : 3 + W])
            r5s = sb.tile([128, W], F16, tag="r5s")
            nc.vector.tensor_add(r5s[:], r5a[:], r5b[:])

            w2 = sb.tile([128, W], F16, tag="w2")
            nc.vector.tensor_mul(w2[:], r2s[:], g[2][:])
            w5 = sb.tile([128, W], F16, tag="w5")
            nc.vector.tensor_mul(w5[:], r5s[:], g[5][:])
            w8 = sb.tile([128, W], F16, tag="w8")
            nc.gpsimd.tensor_mul(w8[:], r8s[:], g[8][:])

            # ---- conv = x + W1 + W2 + W4 + W5 + W8 (PE accumulate) ----
            conv_sb = sb.tile([128, W], F32, tag="conv_sb")
            mask_sb = sb.tile([128, W], F32, tag="mask_sb")
            for ci, (c0, n) in enumerate(COLH):
                cp = ps.tile([128, n], F32, tag=f"cp{ci}")
                nc.tensor.matmul(
                    cp[:], wt["w_id"][:], x16[:, 2 + c0 : 2 + c0 + n],
                    start=True, stop=False,
                )
                for wk in (w1, w2, w4, w5):
                    nc.tensor.matmul(
                        cp[:], wt["w_id"][:], wk[:, c0 : c0 + n],
                        start=False, stop=False,
                    )
                nc.tensor.matmul(
                    cp[:], wt["w_id"][:], w8[:, c0 : c0 + n], start=False, stop=True
                )
                nc.scalar.copy(conv_sb[:, c0 : c0 + n], cp[:])
                nc.vector.tensor_scalar(
                    mask_sb[:, c0 : c0 + n], cp[:], 0.5, None, mybir.AluOpType.is_ge
                )

            nc.sync.dma_start(conv_d[r0 : r0 + 128, :], conv_sb[:])
            nc.sync.dma_start(mask_d[r0 : r0 + 128, :], mask_sb[:])

        if repeat == 1:
            _body()
        else:
            with tc.For_i(0, repeat, 1):
                _body()

    nc.compile()
    return nc


def kernel(bev_map: np.ndarray, bev_scale: np.ndarray):
    assert bev_map.shape == (B, 1, H, W) and bev_scale.shape == (B, 1, H, W)
    if "nc" not in _CACHE:
        _CACHE["nc"] = _build()
    nc = _CACHE["nc"]

    consts = _consts()
    in_maps = []
    for c in range(NCORES):
        b, hh = c // 2, c % 2
        padded = np.pad(bev_map[b, 0], ((2, 2), (0, 0)))  # [772, W]
        m = {
            "xin": np.ascontiguousarray(padded[hh * SLAB : hh * SLAB + SLAB + 4]),
            "sin": np.ascontiguousarray(bev_scale[b, 0, hh * SLAB : (hh + 1) * SLAB]),
        }
        m.update(consts)
        in_maps.append(m)

    res = run_bass_kernel_spmd(nc, in_maps, list(range(NCORES))).results

    conv = np.empty((B, 1, H, W), dtype=np.float32)
    mask = np.empty((B, 1, H, W), dtype=np.float32)
    for c in range(NCORES):
        b, hh = c // 2, c % 2
        conv[b, 0, hh * SLAB : (hh + 1) * SLAB] = res[c]["conv"]
        mask[b, 0, hh * SLAB : (hh + 1) * SLAB] = res[c]["mask"]
    return conv, mask
